# revision 41
# baseline (speedup 1.0000x reference)
"""Trainium2 Bass kernel for a CGNS block (GNN message passing).

Math: the reference builds A = a a^T + I (rank-1 + identity), L = D^-1/2 A D^-1/2,
then out = relu(BN(conv1x1(cat[x@A, (L@x^T)^T]))).  Exploiting the rank-1
structure, with a = relu(tanh(w)), S = sum(a), d_n = 1/sqrt(a_n*S + 1),
u = d*a, s0 = x@a, s1 = x@u, the whole block collapses to

  y[:, n] = W1~ x[:, n] + d2[n] * (W2~ x[:, n]) + a[n] v1 + u[n] v2 + b~
  out     = relu(y)

where W~ are the BN-folded conv weights, v1 = W1~ s0, v2 = W2~ s1.  No [N,N]
matrix is ever materialized.

Sharding: 8 cores; core i handles batch b = i//2, half h = i%2 of the N=4096
node dim (2048 columns each).  Each core reads the full x[b] once in
transposed layout (for the s0/s1 reduction, which needs all of N) and its own
half in natural layout (for the main matmuls).  n-chunks are rolled per-core
so that chunks 0..15 are always the core's own half -> identical SPMD program.

On-device layout is output-transposed (n on partitions) so d2/a/u are
per-partition scalars.
"""

import numpy as np

import concourse.bacc as bacc
import concourse.bass as bass
import concourse.tile as tile
from concourse import mybir

FP = mybir.dt.float32
FPR = mybir.dt.float32r
B, C, N = 4, 64, 4096
NH = N // 2          # columns per core
JH = NH // 128       # 16 chunks per core half
JF = N // 128        # 32 chunks full N
BN_EPS = 1e-5


def build_nc():
    # Bacc (not raw Bass): its compile() pipeline legalizes TRN2's
    # one-wait-per-instruction constraint (move_matmul_waits_to_ldweights,
    # generate_event_semaphores) which Tile-emitted multi-waits require.
    nc = bacc.Bacc()
    AF = mybir.ActivationFunctionType
    OP = mybir.AluOpType
    AX = mybir.AxisListType

    # DRAM I/O (per-core shards supplied via in_maps)
    xt = nc.dram_tensor("xt", [128, JF, C], FPR, kind="ExternalInput")
    xh = nc.dram_tensor("xh", [C, NH], FPR, kind="ExternalInput")
    wcol = nc.dram_tensor("wcol", [128, 32], FP, kind="ExternalInput")
    wrow = nc.dram_tensor("wrow", [JH, 128], FP, kind="ExternalInput")
    wv = nc.dram_tensor("wv", [C, 2 * C], FPR, kind="ExternalInput")
    brow3 = nc.dram_tensor("brow3", [3, 2 * C], FPR, kind="ExternalInput")
    out = nc.dram_tensor("out", [128, JH, C], FP, kind="ExternalOutput")

    with tile.TileContext(nc) as tc:
        with (
            tc.tile_pool(name="sb", bufs=1) as sb,
            tc.tile_pool(name="ps", bufs=1, space="PSUM") as ps,
        ):
            # SBUF tiles
            xt_sb = sb.tile([128, JF, C], FPR, name="xt_sb")
            xa = sb.tile([67, NH], FPR, name="xa")      # x half + a/u/ones rows
            wcol_sb = sb.tile([128, 32], FP, name="wcol_sb")
            wrow_sb = sb.tile([JH, 128], FP, name="wrow_sb")
            trow = sb.tile([JH, 128], FP, name="trow")
            arow = sb.tile([JH, 128], FPR, name="arow")
            drow = sb.tile([JH, 128], FP, name="drow")
            urow = sb.tile([JH, 128], FPR, name="urow")
            wAB = sb.tile([67, 2 * C], FPR, name="wAB")  # [W1~T|W2~T] + v/b~ rows
            orow = sb.tile([JH, 128], FP, name="orow")  # ones row source
            jnk = sb.tile([1, 1], FP, name="jnk")
            ones = sb.tile([128, 128], FP, name="ones")
            tcol = sb.tile([128, 32], FP, name="tcol")
            acol = sb.tile([128, 32], FP, name="acol")
            dcol = sb.tile([128, 32], FP, name="dcol")
            ucol = sb.tile([128, 32], FP, name="ucol")
            d2col = sb.tile([128, 32], FP, name="d2col")
            apart = sb.tile([128, 1], FP, name="apart")
            sS = sb.tile([128, 1], FP, name="sS")
            au = sb.tile([128, 2 * 32], FPR, name="au")  # a/u interleaved
            s01 = sb.tile([C, 2], FPR, name="s01")
            s0z = sb.tile([C, 4], FPR, name="s0z")
            vtmp = sb.tile([2, 2 * C], FPR, name="vtmp")
            y1 = sb.tile([128, JH * C], FP, name="y1")
            yo = sb.tile([128, JH * C], FP, name="yo")

            # PSUM tiles (each padded to a bank; 7 total <= 8 banks)
            p_sm = ps.tile([128, 1], FP, name="p_sm")
            p_s = ps.tile([C, 2], FP, name="p_s")
            p_v = ps.tile([2, 2 * C], FP, name="p_v")
            p_yq = [ps.tile([128, 512], FP, name=f"p_yq_{g}") for g in range(4)]

            # ---- DMAs in (order matters: small params, then xt, then xh) ----
            # Small params first on SP (their completion sems gate the scalar
            # chain and the v-matmul); the big x streams are spread across the
            # SP and Activation HWDGE queues so transfers run concurrently.
            nc.sync.dma_start(wcol_sb[:], wcol[:])
            xt_q = [nc.scalar, nc.sync, nc.scalar, nc.sync]
            for p in range(4):
                xt_q[p].dma_start(
                    xt_sb[:, 8 * p : 8 * (p + 1), :], xt[:, 8 * p : 8 * (p + 1), :]
                )
            nc.sync.dma_start(wAB[0:C, :], wv[:])
            nc.sync.dma_start(wAB[64:67, :], brow3[:])
            nc.sync.dma_start(wrow_sb[:], wrow[:])
            # xh on the SWDGE queue: no source dependency, so gpsimd issues
            # it immediately and the SP queue (xt + params) drains early --
            # every SP completion sem posts sooner (drain-batched sems).
            nc.gpsimd.dma_start(xa[0:C, 0:1024], xh[:, 0:1024])
            nc.gpsimd.dma_start(xa[0:C, 1024:2048], xh[:, 1024:2048])

            # ---- small vector phase: a, S, d, u, d2 in column layout, and
            # a/u in row layout for xa rows 64/65.  ACT ops are grouped
            # Tanh,Tanh then Sqrt,Sqrt so each LUT loads exactly once.
            nc.vector.memset(ones[:], 1.0)
            nc.vector.memset(orow[:], 1.0)
            # ones row 66 of xa via flatten-DMA (engine writes at partition 66
            # would need alignment games; DMA has no partition restrictions)
            nc.gpsimd.dma_start(xa[66:67, :], orow[:].bitcast(FPR))
            # Column path strictly first on every engine: it gates the s0
            # matmuls.  The row path (xa rows 64/65, needed only by the main
            # matmuls much later) fills engine idle time behind it.
            nc.scalar.activation(tcol[:], wcol_sb[:], AF.Tanh)
            nc.vector.tensor_scalar_max(acol[:], tcol[:], 0.0)
            nc.vector.tensor_reduce(apart[:], acol[:], axis=AX.X, op=OP.add)
            nc.scalar.activation(trow[:], wrow_sb[:], AF.Tanh)
            # S broadcast to all partitions via ones matmul
            nc.tensor.matmul(p_sm[:], ones[:], apart[:], start=True, stop=True)
            nc.vector.tensor_copy(sS[:], p_sm[:])
            # t = a*S + 1 ; d2 = 1/t ; d = sqrt(d2) ; u = d*a
            nc.vector.tensor_scalar(tcol[:], acol[:], sS[:], 1.0, op0=OP.mult, op1=OP.add)
            nc.vector.reciprocal(d2col[:], tcol[:])
            nc.scalar.sqrt(dcol[:], d2col[:])
            nc.vector.tensor_mul(ucol[:], dcol[:], acol[:])
            # interleave a/u columns: au[:, 2j] = a[:, j], au[:, 2j+1] = u[:, j]
            au_v = au[:].rearrange("p (k t) -> p k t", t=2)
            nc.vector.tensor_copy(au_v[:, :, 0], acol[:])
            nc.vector.tensor_copy(au_v[:, :, 1], ucol[:])
            # row path
            nc.vector.tensor_scalar_max(arow[:], trow[:], 0.0)
            nc.vector.tensor_scalar(
                trow[:], arow[:], sS[0:JH, :], 1.0, op0=OP.mult, op1=OP.add
            )
            nc.vector.reciprocal(trow[:], trow[:])
            nc.scalar.sqrt(drow[:], trow[:])
            nc.vector.tensor_mul(urow[:], drow[:], arow[:])
            # rows via the otherwise-idle SWDGE queue: they issue the moment
            # urow is ready instead of draining behind 1MB of xh on SP
            nc.gpsimd.dma_start(xa[64:65, :], arow[:])
            nc.gpsimd.dma_start(xa[65:66, :], urow[:])

            # ---- s0/s1 reduction over full N (PE, accumulate in PSUM) ----
            for j in range(JF):
                nc.tensor.matmul(
                    p_s[:],
                    xt_sb[:, j, :],
                    au[:, 2 * j : 2 * j + 2],
                    start=(j == 0),
                    stop=(j == JF - 1),
                )
            nc.vector.tensor_copy(s01[:], p_s[:])

            # v1/v2 on partition 0 side by side, one evacuation, one DMA into
            # wAB rows 64/65 (engine writes at partition 64+ hang HW).
            nc.tensor.matmul(
                p_v[0:1, 0:C], s01[:, 0:1], wAB[0:C, 0:C], start=True, stop=True
            )
            nc.tensor.matmul(
                p_v[0:1, C : 2 * C], s01[:, 1:2], wAB[0:C, C : 2 * C],
                start=True, stop=True,
            )
            nc.vector.tensor_copy(vtmp[0:1, :], p_v[0:1, :])
            nc.sync.dma_start(
                wAB[64:66, 0:C],
                vtmp[0:1, :].rearrange("p (r c) -> p r c", c=C),
            )

            # ---- main matmuls: one [67,128]x[67,128] mm per chunk.
            # out columns 0:64 = y1 (conv1 + rank-1 + bias), 64:128 = q (conv2)
            for j in range(JH):
                grp, jj = divmod(j, 4)
                nc.tensor.matmul(
                    p_yq[grp][:, 128 * jj : 128 * (jj + 1)],
                    xa[:, 128 * j : 128 * (j + 1)],
                    wAB[:],
                    start=True, stop=True,
                )

            # ---- epilogue: yo = relu(q * d2 + y1) ----
            for g in range(4):
                nc.scalar.copy(
                    y1[:, 256 * g : 256 * (g + 1)].rearrange(
                        "p (j c) -> p j c", c=C
                    ),
                    p_yq[g][:].rearrange("p (j c) -> p j c", c=2 * C)[:, :, 0:C],
                )
            for j in range(JH):
                g, jj = divmod(j, 4)
                nc.vector.scalar_tensor_tensor(
                    yo[:, C * j : C * (j + 1)],
                    p_yq[g][:, 128 * jj + C : 128 * jj + 2 * C],
                    d2col[:, j : j + 1],
                    y1[:, C * j : C * (j + 1)],
                    op0=OP.mult,
                    op1=OP.add,
                )
            for g in range(4):
                nc.scalar.activation(
                    yo[:, 256 * g : 256 * (g + 1)], yo[:, 256 * g : 256 * (g + 1)],
                    AF.Relu,
                )
                nc.sync.dma_start(
                    out[:, 4 * g : 4 * (g + 1), :],
                    yo[:, 256 * g : 256 * (g + 1)].rearrange("p (j c) -> p j c", c=C),
                )
    nc.compile()
    return nc


def make_in_maps(x, w, conv_w, conv_b, bn_gamma, bn_beta, bn_mean, bn_var):
    x = np.asarray(x, np.float32)
    w = np.asarray(w, np.float32)
    conv_w = np.asarray(conv_w, np.float32)
    conv_b = np.asarray(conv_b, np.float32)
    bn_gamma = np.asarray(bn_gamma, np.float32)
    bn_beta = np.asarray(bn_beta, np.float32)
    bn_mean = np.asarray(bn_mean, np.float32)
    bn_var = np.asarray(bn_var, np.float32)

    scale = bn_gamma / np.sqrt(bn_var + BN_EPS)
    wmat = conv_w * scale[:, None]                       # [64, 128] BN-folded
    w1t = np.ascontiguousarray(wmat[:, :C].T)            # [c, o]
    w2t = np.ascontiguousarray(wmat[:, C:].T)
    wv = np.ascontiguousarray(np.concatenate([w1t, w2t], axis=1))
    brow3 = np.zeros((3, 2 * C), np.float32)
    brow3[2, :C] = conv_b * scale + bn_beta - bn_mean * scale

    in_maps = []
    for i in range(8):
        b, h = divmod(i, 2)
        xb = x[b, :, :, 0]                               # [64, 4096]
        order = np.roll(np.arange(JF), -JH * h)          # own half first
        xt_jpc = np.ascontiguousarray(xb.T).reshape(JF, 128, C)
        xt_pjc = np.ascontiguousarray(xt_jpc[order].transpose(1, 0, 2))
        xhb = np.ascontiguousarray(xb[:, NH * h : NH * (h + 1)])
        wcol = np.ascontiguousarray(w[b].reshape(JF, 128).T[:, order])
        wrow = np.ascontiguousarray(w[b][NH * h : NH * (h + 1)].reshape(JH, 128))
        in_maps.append(
            {
                "xt": xt_pjc,
                "xh": xhb,
                "wcol": wcol,
                "wrow": wrow,
                "wv": wv,
                "brow3": brow3,
            }
        )
    return in_maps


def assemble_out(results):
    out = np.empty((B, C, N), np.float32)
    for i in range(8):
        b, h = divmod(i, 2)
        blk = np.asarray(results[i]["out"])              # [128, 16, 64]
        y_half = blk.transpose(1, 0, 2).reshape(NH, C)   # row = 128*j + p
        out[b, :, NH * h : NH * (h + 1)] = y_half.T
    return out[..., None]


_NC = None


def kernel(**inputs):
    global _NC
    from concourse.bass_utils import run_bass_kernel_spmd

    if _NC is None:
        _NC = build_nc()
    in_maps = make_in_maps(**inputs)
    res = run_bass_kernel_spmd(_NC, in_maps, list(range(8)))
    return assemble_out(res.results)


# revision 42
# speedup vs baseline: 1.0546x; 1.0546x over previous
"""Trainium2 Bass kernel for a CGNS block (GNN message passing).

Math: the reference builds A = a a^T + I (rank-1 + identity), L = D^-1/2 A D^-1/2,
then out = relu(BN(conv1x1(cat[x@A, (L@x^T)^T]))).  Exploiting the rank-1
structure, with a = relu(tanh(w)), S = sum(a), d_n = 1/sqrt(a_n*S + 1),
u = d*a, s0 = x@a, s1 = x@u, the whole block collapses to

  y[:, n] = W1~ x[:, n] + d2[n] * (W2~ x[:, n]) + a[n] v1 + u[n] v2 + b~
  out     = relu(y)

where W~ are the BN-folded conv weights, v1 = W1~ s0, v2 = W2~ s1.  No [N,N]
matrix is ever materialized.

Sharding: 8 cores; core i handles batch b = i//2, half h = i%2 of the N=4096
node dim (2048 columns each).  Each core reads the full x[b] once in
transposed layout (for the s0/s1 reduction, which needs all of N) and its own
half in natural layout (for the main matmuls).  n-chunks are rolled per-core
so that chunks 0..15 are always the core's own half -> identical SPMD program.

On-device layout is output-transposed (n on partitions) so d2/a/u are
per-partition scalars.
"""

import numpy as np

import concourse.bacc as bacc
import concourse.bass as bass
import concourse.tile as tile
from concourse import mybir

FP = mybir.dt.float32
FPR = mybir.dt.float32r
B, C, N = 4, 64, 4096
NH = N // 2          # columns per core
JH = NH // 128       # 16 chunks per core half
JF = N // 128        # 32 chunks full N
BN_EPS = 1e-5


def build_nc():
    # Bacc (not raw Bass): its compile() pipeline legalizes TRN2's
    # one-wait-per-instruction constraint (move_matmul_waits_to_ldweights,
    # generate_event_semaphores) which Tile-emitted multi-waits require.
    nc = bacc.Bacc()
    AF = mybir.ActivationFunctionType
    OP = mybir.AluOpType
    AX = mybir.AxisListType

    # DRAM I/O (per-core shards supplied via in_maps)
    xt = nc.dram_tensor("xt", [128, JF, C], FPR, kind="ExternalInput")
    xh = nc.dram_tensor("xh", [C, NH], FPR, kind="ExternalInput")
    wcol = nc.dram_tensor("wcol", [128, 32], FP, kind="ExternalInput")
    wrow = nc.dram_tensor("wrow", [JH, 128], FP, kind="ExternalInput")
    wv = nc.dram_tensor("wv", [C, 2 * C], FPR, kind="ExternalInput")
    brow3 = nc.dram_tensor("brow3", [3, 2 * C], FPR, kind="ExternalInput")
    out = nc.dram_tensor("out", [128, JH, C], FP, kind="ExternalOutput")

    with tile.TileContext(nc) as tc:
        with (
            tc.tile_pool(name="sb", bufs=1) as sb,
            tc.tile_pool(name="ps", bufs=1, space="PSUM") as ps,
        ):
            # SBUF tiles
            xt_sb = sb.tile([128, JF, C], FPR, name="xt_sb")
            xa = sb.tile([67, NH], FPR, name="xa")      # x half + a/u/ones rows
            wcol_sb = sb.tile([128, 32], FP, name="wcol_sb")
            wrow_sb = sb.tile([JH, 128], FP, name="wrow_sb")
            trow = sb.tile([JH, 128], FP, name="trow")
            arow = sb.tile([JH, 128], FPR, name="arow")
            drow = sb.tile([JH, 128], FP, name="drow")
            urow = sb.tile([JH, 128], FPR, name="urow")
            wAB = sb.tile([67, 2 * C], FPR, name="wAB")  # [W1~T|W2~T] + v/b~ rows
            orow = sb.tile([JH, 128], FP, name="orow")  # ones row source
            jnk = sb.tile([1, 1], FP, name="jnk")
            ones = sb.tile([128, 128], FP, name="ones")
            tcol = sb.tile([128, 32], FP, name="tcol")
            acol = sb.tile([128, 32], FP, name="acol")
            dcol = sb.tile([128, 32], FP, name="dcol")
            ucol = sb.tile([128, 32], FP, name="ucol")
            d2col = sb.tile([128, 32], FP, name="d2col")
            apart = sb.tile([128, 1], FP, name="apart")
            sS = sb.tile([128, 1], FP, name="sS")
            au = sb.tile([128, 2 * 32], FPR, name="au")  # a/u interleaved
            s01 = sb.tile([C, 2], FPR, name="s01")
            s0z = sb.tile([C, 4], FPR, name="s0z")
            vtmp = sb.tile([2, 2 * C], FPR, name="vtmp")
            y1 = sb.tile([128, JH * C], FP, name="y1")
            yo = sb.tile([128, JH * C], FP, name="yo")

            # PSUM tiles (each padded to a bank; 7 total <= 8 banks)
            p_sm = ps.tile([128, 1], FP, name="p_sm")
            p_s = ps.tile([C, 2], FP, name="p_s")
            p_v = ps.tile([2, 2 * C], FP, name="p_v")
            p_yq = [ps.tile([128, 512], FP, name=f"p_yq_{g}") for g in range(4)]

            # ---- DMAs in (order matters: small params, then xt, then xh) ----
            # Small params first on SP (their completion sems gate the scalar
            # chain and the v-matmul); the big x streams are spread across the
            # SP and Activation HWDGE queues so transfers run concurrently.
            nc.sync.dma_start(wcol_sb[:], wcol[:])
            xt_q = [nc.scalar, nc.sync, nc.scalar, nc.sync]
            for p in range(4):
                xt_q[p].dma_start(
                    xt_sb[:, 8 * p : 8 * (p + 1), :], xt[:, 8 * p : 8 * (p + 1), :]
                )
            nc.sync.dma_start(wAB[0:C, :], wv[:])
            nc.sync.dma_start(wAB[64:67, :], brow3[:])
            nc.sync.dma_start(wrow_sb[:], wrow[:])
            nc.sync.dma_start(xa[0:C, 0:1024], xh[:, 0:1024])
            nc.sync.dma_start(xa[0:C, 1024:2048], xh[:, 1024:2048])

            # ---- small vector phase: a, S, d, u, d2 in column layout, and
            # a/u in row layout for xa rows 64/65.  ACT ops are grouped
            # Tanh,Tanh then Sqrt,Sqrt so each LUT loads exactly once.
            nc.vector.memset(ones[:], 1.0)
            nc.vector.memset(orow[:], 1.0)
            # ones row 66 of xa via flatten-DMA (engine writes at partition 66
            # would need alignment games; DMA has no partition restrictions)
            nc.gpsimd.dma_start(xa[66:67, :], orow[:].bitcast(FPR))
            # Column path strictly first on every engine: it gates the s0
            # matmuls.  The row path (xa rows 64/65, needed only by the main
            # matmuls much later) fills engine idle time behind it.
            nc.scalar.activation(tcol[:], wcol_sb[:], AF.Tanh)
            nc.vector.tensor_scalar_max(acol[:], tcol[:], 0.0)
            nc.vector.tensor_reduce(apart[:], acol[:], axis=AX.X, op=OP.add)
            nc.scalar.activation(trow[:], wrow_sb[:], AF.Tanh)
            # S broadcast to all partitions via ones matmul
            nc.tensor.matmul(p_sm[:], ones[:], apart[:], start=True, stop=True)
            nc.vector.tensor_copy(sS[:], p_sm[:])
            # t = a*S + 1 ; d2 = 1/t ; d = sqrt(d2) ; u = d*a
            nc.vector.tensor_scalar(tcol[:], acol[:], sS[:], 1.0, op0=OP.mult, op1=OP.add)
            nc.vector.reciprocal(d2col[:], tcol[:])
            nc.scalar.sqrt(dcol[:], d2col[:])
            nc.vector.tensor_mul(ucol[:], dcol[:], acol[:])
            # interleave a/u columns: au[:, 2j] = a[:, j], au[:, 2j+1] = u[:, j]
            au_v = au[:].rearrange("p (k t) -> p k t", t=2)
            nc.vector.tensor_copy(au_v[:, :, 0], acol[:])
            nc.vector.tensor_copy(au_v[:, :, 1], ucol[:])
            # row path
            nc.vector.tensor_scalar_max(arow[:], trow[:], 0.0)
            nc.vector.tensor_scalar(
                trow[:], arow[:], sS[0:JH, :], 1.0, op0=OP.mult, op1=OP.add
            )
            nc.vector.reciprocal(trow[:], trow[:])
            nc.scalar.sqrt(drow[:], trow[:])
            nc.vector.tensor_mul(urow[:], drow[:], arow[:])
            # rows via the otherwise-idle SWDGE queue: they issue the moment
            # urow is ready instead of draining behind 1MB of xh on SP
            nc.gpsimd.dma_start(xa[64:65, :], arow[:])
            nc.gpsimd.dma_start(xa[65:66, :], urow[:])

            # ---- s0/s1 reduction over full N (PE, accumulate in PSUM) ----
            for j in range(JF):
                nc.tensor.matmul(
                    p_s[:],
                    xt_sb[:, j, :],
                    au[:, 2 * j : 2 * j + 2],
                    start=(j == 0),
                    stop=(j == JF - 1),
                )
            nc.vector.tensor_copy(s01[:], p_s[:])

            # v1/v2 on partition 0 side by side, one evacuation, one DMA into
            # wAB rows 64/65 (engine writes at partition 64+ hang HW).
            nc.tensor.matmul(
                p_v[0:1, 0:C], s01[:, 0:1], wAB[0:C, 0:C], start=True, stop=True
            )
            nc.tensor.matmul(
                p_v[0:1, C : 2 * C], s01[:, 1:2], wAB[0:C, C : 2 * C],
                start=True, stop=True,
            )
            nc.vector.tensor_copy(vtmp[0:1, :], p_v[0:1, :])
            nc.sync.dma_start(
                wAB[64:66, 0:C],
                vtmp[0:1, :].rearrange("p (r c) -> p r c", c=C),
            )

            # ---- main matmuls: one [67,128]x[67,128] mm per chunk.
            # out columns 0:64 = y1 (conv1 + rank-1 + bias), 64:128 = q (conv2)
            for j in range(JH):
                grp, jj = divmod(j, 4)
                nc.tensor.matmul(
                    p_yq[grp][:, 128 * jj : 128 * (jj + 1)],
                    xa[:, 128 * j : 128 * (j + 1)],
                    wAB[:],
                    start=True, stop=True,
                )

            # ---- epilogue: yo = relu(q * d2 + y1) ----
            for g in range(4):
                nc.scalar.copy(
                    y1[:, 256 * g : 256 * (g + 1)].rearrange(
                        "p (j c) -> p j c", c=C
                    ),
                    p_yq[g][:].rearrange("p (j c) -> p j c", c=2 * C)[:, :, 0:C],
                )
            for j in range(JH):
                g, jj = divmod(j, 4)
                nc.vector.scalar_tensor_tensor(
                    yo[:, C * j : C * (j + 1)],
                    p_yq[g][:, 128 * jj + C : 128 * jj + 2 * C],
                    d2col[:, j : j + 1],
                    y1[:, C * j : C * (j + 1)],
                    op0=OP.mult,
                    op1=OP.add,
                )
            for g in range(4):
                nc.scalar.activation(
                    yo[:, 256 * g : 256 * (g + 1)], yo[:, 256 * g : 256 * (g + 1)],
                    AF.Relu,
                )
                nc.sync.dma_start(
                    out[:, 4 * g : 4 * (g + 1), :],
                    yo[:, 256 * g : 256 * (g + 1)].rearrange("p (j c) -> p j c", c=C),
                )
    nc.compile()
    return nc


def make_in_maps(x, w, conv_w, conv_b, bn_gamma, bn_beta, bn_mean, bn_var):
    x = np.asarray(x, np.float32)
    w = np.asarray(w, np.float32)
    conv_w = np.asarray(conv_w, np.float32)
    conv_b = np.asarray(conv_b, np.float32)
    bn_gamma = np.asarray(bn_gamma, np.float32)
    bn_beta = np.asarray(bn_beta, np.float32)
    bn_mean = np.asarray(bn_mean, np.float32)
    bn_var = np.asarray(bn_var, np.float32)

    scale = bn_gamma / np.sqrt(bn_var + BN_EPS)
    wmat = conv_w * scale[:, None]                       # [64, 128] BN-folded
    w1t = np.ascontiguousarray(wmat[:, :C].T)            # [c, o]
    w2t = np.ascontiguousarray(wmat[:, C:].T)
    wv = np.ascontiguousarray(np.concatenate([w1t, w2t], axis=1))
    brow3 = np.zeros((3, 2 * C), np.float32)
    brow3[2, :C] = conv_b * scale + bn_beta - bn_mean * scale

    in_maps = []
    for i in range(8):
        b, h = divmod(i, 2)
        xb = x[b, :, :, 0]                               # [64, 4096]
        order = np.roll(np.arange(JF), -JH * h)          # own half first
        xt_jpc = np.ascontiguousarray(xb.T).reshape(JF, 128, C)
        xt_pjc = np.ascontiguousarray(xt_jpc[order].transpose(1, 0, 2))
        xhb = np.ascontiguousarray(xb[:, NH * h : NH * (h + 1)])
        wcol = np.ascontiguousarray(w[b].reshape(JF, 128).T[:, order])
        wrow = np.ascontiguousarray(w[b][NH * h : NH * (h + 1)].reshape(JH, 128))
        in_maps.append(
            {
                "xt": xt_pjc,
                "xh": xhb,
                "wcol": wcol,
                "wrow": wrow,
                "wv": wv,
                "brow3": brow3,
            }
        )
    return in_maps


def assemble_out(results):
    out = np.empty((B, C, N), np.float32)
    for i in range(8):
        b, h = divmod(i, 2)
        blk = np.asarray(results[i]["out"])              # [128, 16, 64]
        y_half = blk.transpose(1, 0, 2).reshape(NH, C)   # row = 128*j + p
        out[b, :, NH * h : NH * (h + 1)] = y_half.T
    return out[..., None]


_NC = None


def kernel(**inputs):
    global _NC
    from concourse.bass_utils import run_bass_kernel_spmd

    if _NC is None:
        _NC = build_nc()
    in_maps = make_in_maps(**inputs)
    res = run_bass_kernel_spmd(_NC, in_maps, list(range(8)))
    return assemble_out(res.results)
